# revision 15
# baseline (speedup 1.0000x reference)
"""RPN training-proposal kernel for Trainium2 (8 NeuronCores, SPMD row-parallel).

Device (per core, anchors row-sharded): for each of 64 gt boxes, computes
J = inter/(area_a + area_g + 1e-8) per anchor — a monotone transform of IoU
(iou = J/(1-J)) — tracking max-over-gt J per anchor (labels) and max-over-
anchor J per gt (per-gt argmax location). Host: threshold labeling, exact
tiny recomputes for the <=1.5k selected/forced anchors, and the 256-sample
top-k selection using the fixed PRNG streams.
"""
import numpy as np

import concourse.bass as bass
import concourse.mybir as mybir
from concourse.bass_utils import run_bass_kernel_spmd

N, G = 500000, 64
NC = 8
SHARD = N // NC           # 62500
P, C = 128, 489           # P*C = 62592 >= SHARD
PAD = P * C - SHARD
TOTAL_SAMPLES, MAX_POS = 256, 128
NEG_T = 0.995             # conservative rn threshold for neg top-k candidates

f32 = np.float32
_PROG_CACHE = {}
_RAND_CACHE = {}


def _get_rand():
    if "rp" not in _RAND_CACHE:
        import jax
        cpu = jax.devices("cpu")[0]
        with jax.default_device(cpu):
            kp, kn = jax.random.split(jax.random.key(42))
            rp = np.asarray(jax.random.uniform(kp, (N,)))
            rn = np.asarray(jax.random.uniform(kn, (N,)))
        _RAND_CACHE["rp"] = rp
        _RAND_CACHE["rn"] = rn
        _RAND_CACHE["hi"] = np.flatnonzero(rn > NEG_T)
    return _RAND_CACHE["rp"], _RAND_CACHE["rn"], _RAND_CACHE["hi"]


Q7 = 0.7 / 1.7
Q3 = 0.3 / 1.3


def _build_program(gt):
    """Bass program with gt constants baked as immediates. gt: [G,4] f32.

    Per gt: inter[p,c] f32-exact; label predicates accumulate
    acc_q = max_g(inter - q*cg) (host compares vs q*area); per-gt argmax
    location via colmax[:,g] = max_c(ln(inter+1e-10) - ln(area+cg)) — a
    monotone transform of iou within the column.
    """
    dt = mybir.dt.float32
    nc = bass.Bass()
    names = ["aymax", "naymin", "axmax", "naxmin", "area"]
    dins = {n: nc.dram_tensor(n, [P, C], dt, kind="ExternalInput") for n in names}
    d_p7 = nc.dram_tensor("p7", [P, C], dt, kind="ExternalOutput")
    d_p3 = nc.dram_tensor("p3", [P, C], dt, kind="ExternalOutput")
    d_colmax = nc.dram_tensor("colmax", [P, G], dt, kind="ExternalOutput")

    gy1 = [float(gt[g, 0]) for g in range(G)]
    gx1 = [float(gt[g, 1]) for g in range(G)]
    gy2 = [float(gt[g, 2]) for g in range(G)]
    gx2 = [float(gt[g, 3]) for g in range(G)]
    cg = [float((f32(gt[g, 2] - gt[g, 0]) * f32(gt[g, 3] - gt[g, 1])).astype(f32)
                + f32(1e-8)) for g in range(G)]

    import contextlib
    with contextlib.ExitStack() as ctx:
        # [P,1] bias columns for the ACT Ln ops: col 0 = 1e-10, col 1+g = cg[g]
        bias_t = ctx.enter_context(nc.sbuf_tensor("ln_biases", [P, G + 1], dt))
        nc.gpsimd.memset(bias_t[:, 0:1], 1e-10)
        for g in range(G):
            nc.gpsimd.memset(bias_t[:, g + 1:g + 2], cg[g])
        nc.all_engine_barrier()
        sb = {n: ctx.enter_context(nc.sbuf_tensor("sb_" + n, [P, C], dt))
              for n in names}
        t1 = ctx.enter_context(nc.sbuf_tensor("t1", [P, C], dt))
        hy = ctx.enter_context(nc.sbuf_tensor("hy", [P, C], dt))
        hyr = ctx.enter_context(nc.sbuf_tensor("hyr", [P, C], dt))
        wx = ctx.enter_context(nc.sbuf_tensor("wx", [P, C], dt))
        inter = ctx.enter_context(nc.sbuf_tensor("inter", [P, C], dt))
        acc7 = ctx.enter_context(nc.sbuf_tensor("acc7", [P, C], dt))
        acc3 = ctx.enter_context(nc.sbuf_tensor("acc3", [P, C], dt))
        lnI = [ctx.enter_context(nc.sbuf_tensor(f"lnI{i}", [P, C], dt))
               for i in range(2)]
        lnD = [ctx.enter_context(nc.sbuf_tensor(f"lnD{i}", [P, C], dt))
               for i in range(2)]
        colv = ctx.enter_context(nc.sbuf_tensor("colv", [P, C], dt))
        colmax = ctx.enter_context(nc.sbuf_tensor("colmax_sb", [P, G], dt))
        dma_sem = ctx.enter_context(nc.semaphore("dma_sem"))
        v_sem = ctx.enter_context(nc.semaphore("v_sem"))
        a_sem = ctx.enter_context(nc.semaphore("a_sem"))
        f_sem = ctx.enter_context(nc.semaphore("f_sem"))
        block = ctx.enter_context(nc.Block())

        op = mybir.AluOpType
        AF = mybir.ActivationFunctionType

        @block.sync
        def _(sync):
            for n in names:
                sync.dma_start(sb[n][:], dins[n][:]).then_inc(dma_sem, 16)
            sync.wait_ge(f_sem, 1)
            sync.dma_start(d_p7[:], acc7[:]).then_inc(dma_sem, 16)
            sync.dma_start(d_p3[:], acc3[:]).then_inc(dma_sem, 16)
            sync.dma_start(d_colmax[:], colmax[:]).then_inc(dma_sem, 16)
            sync.wait_ge(dma_sem, 8 * 16)

        @block.scalar
        def _(scalar):
            for g in range(G):
                scalar.wait_ge(v_sem, g + 1)
                scalar.activation(lnI[g % 2][:], inter[:], AF.Ln,
                                  bias=bias_t[:, 0:1]).then_inc(a_sem, 1)
                scalar.activation(lnD[g % 2][:], sb["area"][:], AF.Ln,
                                  bias=bias_t[:, g + 1:g + 2]).then_inc(a_sem, 1)

        @block.vector
        def _(vector):
            vector.memset(acc7[:], -1e30)
            vector.memset(acc3[:], -1e30)
            vector.wait_ge(dma_sem, 5 * 16)
            for g in range(G):
                vector.tensor_scalar_min(t1[:], sb["aymax"][:], gy2[g])
                vector.scalar_tensor_tensor(hy[:], sb["naymin"][:], -gy1[g], t1[:],
                                            op.min, op.add)
                vector.tensor_scalar_max(hyr[:], hy[:], 0.0)
                vector.tensor_scalar_min(t1[:], sb["axmax"][:], gx2[g])
                vector.scalar_tensor_tensor(wx[:], sb["naxmin"][:], -gx1[g], t1[:],
                                            op.min, op.add)
                if g > 0:
                    # inter(g) overwrite must wait for ACT lnI(g-1) read
                    vector.wait_ge(a_sem, 2 * g - 1)
                vector.scalar_tensor_tensor(inter[:], wx[:], 0.0, hyr[:],
                                            op.max, op.mult)
                vector.scalar_tensor_tensor(acc7[:], inter[:], -Q7 * cg[g],
                                            acc7[:], op.add, op.max)
                vector.scalar_tensor_tensor(acc3[:], inter[:], -Q3 * cg[g],
                                            acc3[:], op.add,
                                            op.max).then_inc(v_sem, 1)
                if g > 0:
                    vector.wait_ge(a_sem, 2 * g)
                    vector.tensor_tensor(colv[:], lnI[(g - 1) % 2][:],
                                         lnD[(g - 1) % 2][:], op.subtract)
                    vector.tensor_reduce(colmax[:, g - 1:g], colv[:],
                                         mybir.AxisListType.X, op.max)
            g = G - 1
            vector.wait_ge(a_sem, 2 * G)
            vector.tensor_tensor(colv[:], lnI[g % 2][:], lnD[g % 2][:],
                                 op.subtract)
            vector.tensor_reduce(colmax[:, g:g + 1], colv[:],
                                 mybir.AxisListType.X, op.max)
            # fold q*area into the accumulators: p = acc - q*area
            vector.scalar_tensor_tensor(acc7[:], sb["area"][:], -Q7, acc7[:],
                                        op.mult, op.add)
            vector.scalar_tensor_tensor(acc3[:], sb["area"][:], -Q3, acc3[:],
                                        op.mult, op.add).then_inc(f_sem, 1)

    return nc


N_DVE_GTS = 35  # gts 0..34 on the vector engine, 35..63 on gpsimd


def _build_program_v2(gt):
    """3-engine split: DVE and GPSIMD each own a gt subset (full predicate
    chain + colv); ACT computes the two Ln's per gt; DVE does all column
    reduces and the final merge."""
    dt = mybir.dt.float32
    nc = bass.Bass()
    names = ["aymax", "naymin", "axmax", "naxmin", "area"]
    dins = {n: nc.dram_tensor(n, [P, C], dt, kind="ExternalInput") for n in names}
    d_p7 = nc.dram_tensor("p7", [P, C], dt, kind="ExternalOutput")
    d_p3 = nc.dram_tensor("p3", [P, C], dt, kind="ExternalOutput")
    d_colmax = nc.dram_tensor("colmax", [P, G], dt, kind="ExternalOutput")

    gy1 = [float(gt[g, 0]) for g in range(G)]
    gx1 = [float(gt[g, 1]) for g in range(G)]
    gy2 = [float(gt[g, 2]) for g in range(G)]
    gx2 = [float(gt[g, 3]) for g in range(G)]
    cg = [float((f32(gt[g, 2] - gt[g, 0]) * f32(gt[g, 3] - gt[g, 1])).astype(f32)
                + f32(1e-8)) for g in range(G)]
    DG = list(range(N_DVE_GTS))
    SG = list(range(N_DVE_GTS, G))

    import contextlib
    with contextlib.ExitStack() as ctx:
        bias_t = ctx.enter_context(nc.sbuf_tensor("ln_biases", [P, G + 1], dt))
        nc.gpsimd.memset(bias_t[:, 0:1], 1e-10)
        for g in range(G):
            nc.gpsimd.memset(bias_t[:, g + 1:g + 2], cg[g])
        nc.all_engine_barrier()
        sb = {n: ctx.enter_context(nc.sbuf_tensor("sb_" + n, [P, C], dt))
              for n in names}

        def sbufs(prefix, num=1):
            if num == 1:
                return ctx.enter_context(nc.sbuf_tensor(prefix, [P, C], dt))
            return [ctx.enter_context(nc.sbuf_tensor(f"{prefix}{i}", [P, C], dt))
                    for i in range(num)]

        t1D, hyD, hyrD, wxD, interD = (sbufs(x) for x in
                                       ("t1D", "hyD", "hyrD", "wxD", "interD"))
        t1S, hyS, hyrS, wxS, interS = (sbufs(x) for x in
                                       ("t1S", "hyS", "hyrS", "wxS", "interS"))
        acc7D, acc3D, acc7S, acc3S = (sbufs(x) for x in
                                      ("acc7D", "acc3D", "acc7S", "acc3S"))
        lnID, lnDD = sbufs("lnID", 2), sbufs("lnDD", 2)
        lnIS, lnDS = sbufs("lnIS", 2), sbufs("lnDS", 2)
        colvD = sbufs("colvD")
        colvS = sbufs("colvS", len(SG))
        colmax = ctx.enter_context(nc.sbuf_tensor("colmax_sb", [P, G], dt))
        dma_sem = ctx.enter_context(nc.semaphore("dma_sem"))
        vD = ctx.enter_context(nc.semaphore("vD"))
        vS = ctx.enter_context(nc.semaphore("vS"))
        aD = ctx.enter_context(nc.semaphore("aD"))
        aS = ctx.enter_context(nc.semaphore("aS"))
        cS = ctx.enter_context(nc.semaphore("cS"))
        g_done = ctx.enter_context(nc.semaphore("g_done"))
        f_sem = ctx.enter_context(nc.semaphore("f_sem"))
        block = ctx.enter_context(nc.Block())

        op = mybir.AluOpType
        AF = mybir.ActivationFunctionType

        @block.sync
        def _(sync):
            for n in names:
                sync.dma_start(sb[n][:], dins[n][:]).then_inc(dma_sem, 16)
            sync.wait_ge(f_sem, 1)
            sync.dma_start(d_p7[:], acc7D[:]).then_inc(dma_sem, 16)
            sync.dma_start(d_p3[:], acc3D[:]).then_inc(dma_sem, 16)
            sync.dma_start(d_colmax[:], colmax[:]).then_inc(dma_sem, 16)
            sync.wait_ge(dma_sem, 8 * 16)

        @block.scalar
        def _(scalar):
            for i in range(max(len(DG), len(SG))):
                if i < len(DG):
                    g = DG[i]
                    scalar.wait_ge(vD, i + 1)
                    scalar.activation(lnID[i % 2][:], interD[:], AF.Ln,
                                      bias=bias_t[:, 0:1]).then_inc(aD, 1)
                    scalar.activation(lnDD[i % 2][:], sb["area"][:], AF.Ln,
                                      bias=bias_t[:, g + 1:g + 2]).then_inc(aD, 1)
                if i < len(SG):
                    g = SG[i]
                    scalar.wait_ge(vS, i + 1)
                    scalar.activation(lnIS[i % 2][:], interS[:], AF.Ln,
                                      bias=bias_t[:, 0:1]).then_inc(aS, 1)
                    scalar.activation(lnDS[i % 2][:], sb["area"][:], AF.Ln,
                                      bias=bias_t[:, g + 1:g + 2]).then_inc(aS, 1)

        def chain(eng, i, g, t1, hy, hyr, wx, inter, acc7, acc3, a_sem, v_sem):
            eng.tensor_scalar_min(t1[:], sb["aymax"][:], gy2[g])
            eng.scalar_tensor_tensor(hy[:], sb["naymin"][:], -gy1[g], t1[:],
                                     op.min, op.add)
            eng.tensor_scalar_max(hyr[:], hy[:], 0.0)
            eng.tensor_scalar_min(t1[:], sb["axmax"][:], gx2[g])
            eng.scalar_tensor_tensor(wx[:], sb["naxmin"][:], -gx1[g], t1[:],
                                     op.min, op.add)
            if i > 0:
                eng.wait_ge(a_sem, 2 * i - 1)
            eng.scalar_tensor_tensor(inter[:], wx[:], 0.0, hyr[:],
                                     op.max, op.mult)
            eng.scalar_tensor_tensor(acc7[:], inter[:], -Q7 * cg[g], acc7[:],
                                     op.add, op.max)
            eng.scalar_tensor_tensor(acc3[:], inter[:], -Q3 * cg[g], acc3[:],
                                     op.add, op.max).then_inc(v_sem, 1)

        @block.gpsimd
        def _(gpsimd):
            gpsimd.memset(acc7S[:], -1e30)
            gpsimd.memset(acc3S[:], -1e30)
            gpsimd.wait_ge(dma_sem, 5 * 16)
            for i, g in enumerate(SG):
                chain(gpsimd, i, g, t1S, hyS, hyrS, wxS, interS,
                      acc7S, acc3S, aS, vS)
                if i > 0:
                    gpsimd.wait_ge(aS, 2 * i)
                    gpsimd.scalar_tensor_tensor(
                        colvS[i - 1][:], lnIS[(i - 1) % 2][:], 0.0,
                        lnDS[(i - 1) % 2][:], op.add,
                        op.subtract).then_inc(cS, 1)
            i = len(SG) - 1
            gpsimd.wait_ge(aS, 2 * len(SG))
            gpsimd.scalar_tensor_tensor(
                colvS[i][:], lnIS[i % 2][:], 0.0, lnDS[i % 2][:],
                op.add, op.subtract).then_inc(cS, 1)
            gpsimd.engine_nop().then_inc(g_done, 1)

        @block.vector
        def _(vector):
            vector.memset(acc7D[:], -1e30)
            vector.memset(acc3D[:], -1e30)
            vector.wait_ge(dma_sem, 5 * 16)
            for i, g in enumerate(DG):
                chain(vector, i, g, t1D, hyD, hyrD, wxD, interD,
                      acc7D, acc3D, aD, vD)
                if i > 0:
                    vector.wait_ge(aD, 2 * i)
                    vector.tensor_tensor(colvD[:], lnID[(i - 1) % 2][:],
                                         lnDD[(i - 1) % 2][:], op.subtract)
                    vector.tensor_reduce(colmax[:, DG[i - 1]:DG[i - 1] + 1],
                                         colvD[:], mybir.AxisListType.X, op.max)
            i = len(DG) - 1
            vector.wait_ge(aD, 2 * len(DG))
            vector.tensor_tensor(colvD[:], lnID[i % 2][:], lnDD[i % 2][:],
                                 op.subtract)
            vector.tensor_reduce(colmax[:, DG[i]:DG[i] + 1], colvD[:],
                                 mybir.AxisListType.X, op.max)
            for j in range(len(SG)):
                vector.wait_ge(cS, j + 1)
                vector.tensor_reduce(colmax[:, SG[j]:SG[j] + 1],
                                     colvS[j][:], mybir.AxisListType.X, op.max)
            vector.wait_ge(g_done, 1)
            vector.tensor_tensor(acc7D[:], acc7D[:], acc7S[:], op.max)
            vector.tensor_tensor(acc3D[:], acc3D[:], acc3S[:], op.max)
            vector.scalar_tensor_tensor(acc7D[:], sb["area"][:], -Q7, acc7D[:],
                                        op.mult, op.add)
            vector.scalar_tensor_tensor(acc3D[:], sb["area"][:], -Q3, acc3D[:],
                                        op.mult, op.add).then_inc(f_sem, 1)

    return nc


def _build_program_v3(L):
    """Culled variant: L gt-slots per core, per-slot constants supplied via a
    [P, 7L+1] input ("consts"), so the program depends only on L. Slot l
    columns: [gy2, -gy1, gx2, -gx1, -q7*cg, -q3*cg, cg]; column 7L = 1e-10
    (the Ln epsilon)."""
    dt = mybir.dt.float32
    nc = bass.Bass()
    names = ["aymax", "naymin", "axmax", "naxmin", "area"]
    dins = {n: nc.dram_tensor(n, [P, C], dt, kind="ExternalInput") for n in names}
    d_consts = nc.dram_tensor("consts", [P, 7 * L + 1], dt, kind="ExternalInput")
    d_p7 = nc.dram_tensor("p7", [P, C], dt, kind="ExternalOutput")
    d_p3 = nc.dram_tensor("p3", [P, C], dt, kind="ExternalOutput")
    d_colmax = nc.dram_tensor("colmax", [P, L], dt, kind="ExternalOutput")

    import contextlib
    with contextlib.ExitStack() as ctx:
        ko = ctx.enter_context(nc.sbuf_tensor("ko", [P, 7 * L + 1], dt))
        sb = {n: ctx.enter_context(nc.sbuf_tensor("sb_" + n, [P, C], dt))
              for n in names}
        t1 = ctx.enter_context(nc.sbuf_tensor("t1", [P, C], dt))
        hy = ctx.enter_context(nc.sbuf_tensor("hy", [P, C], dt))
        hyr = ctx.enter_context(nc.sbuf_tensor("hyr", [P, C], dt))
        wx = ctx.enter_context(nc.sbuf_tensor("wx", [P, C], dt))
        inter = ctx.enter_context(nc.sbuf_tensor("inter", [P, C], dt))
        acc7 = ctx.enter_context(nc.sbuf_tensor("acc7", [P, C], dt))
        acc3 = ctx.enter_context(nc.sbuf_tensor("acc3", [P, C], dt))
        lnI = [ctx.enter_context(nc.sbuf_tensor(f"lnI{i}", [P, C], dt))
               for i in range(2)]
        lnD = [ctx.enter_context(nc.sbuf_tensor(f"lnD{i}", [P, C], dt))
               for i in range(2)]
        colv = ctx.enter_context(nc.sbuf_tensor("colv", [P, C], dt))
        colmax = ctx.enter_context(nc.sbuf_tensor("colmax_sb", [P, L], dt))
        dma_sem = ctx.enter_context(nc.semaphore("dma_sem"))
        v_sem = ctx.enter_context(nc.semaphore("v_sem"))
        a_sem = ctx.enter_context(nc.semaphore("a_sem"))
        hv_sem = ctx.enter_context(nc.semaphore("hv_sem"))
        ar_sem = ctx.enter_context(nc.semaphore("ar_sem"))
        f_sem = ctx.enter_context(nc.semaphore("f_sem"))
        block = ctx.enter_context(nc.Block())

        op = mybir.AluOpType
        AF = mybir.ActivationFunctionType

        def kc(l, j):
            return ko[:, 7 * l + j:7 * l + j + 1]

        @block.sync
        def _(sync):
            sync.dma_start(ko[:], d_consts[:]).then_inc(dma_sem, 16)
            for n in names:
                sync.dma_start(sb[n][:], dins[n][:]).then_inc(dma_sem, 16)
            sync.wait_ge(f_sem, 1)
            sync.dma_start(d_p7[:], acc7[:]).then_inc(dma_sem, 16)
            sync.dma_start(d_p3[:], acc3[:]).then_inc(dma_sem, 16)
            sync.dma_start(d_colmax[:], colmax[:]).then_inc(dma_sem, 16)
            sync.wait_ge(dma_sem, 9 * 16)

        @block.scalar
        def _(scalar):
            for l in range(L):
                scalar.wait_ge(v_sem, l + 1)
                scalar.activation(lnI[l % 2][:], inter[:], AF.Ln,
                                  bias=ko[:, 7 * L:7 * L + 1]).then_inc(a_sem, 1)
                scalar.activation(lnD[l % 2][:], sb["area"][:], AF.Ln,
                                  bias=kc(l, 6)).then_inc(a_sem, 1)

        @block.vector
        def _(vector):
            vector.memset(acc7[:], -1e30)
            vector.memset(acc3[:], -1e30)
            vector.wait_ge(dma_sem, 3 * 16)
            for l in range(L):
                vector.tensor_scalar_min(t1[:], sb["aymax"][:], kc(l, 0))
                vector.scalar_tensor_tensor(hy[:], sb["naymin"][:], kc(l, 1),
                                            t1[:], op.min, op.add)
                vector.tensor_scalar_max(hyr[:], hy[:], 0.0)
                if l == 0:
                    vector.wait_ge(dma_sem, 6 * 16)
                vector.tensor_scalar_min(t1[:], sb["axmax"][:], kc(l, 2))
                vector.scalar_tensor_tensor(wx[:], sb["naxmin"][:], kc(l, 3),
                                            t1[:], op.min, op.add)
                if l > 0:
                    vector.wait_ge(a_sem, 2 * l - 1)
                vector.scalar_tensor_tensor(inter[:], wx[:], 0.0, hyr[:],
                                            op.max, op.mult)
                vector.scalar_tensor_tensor(acc7[:], inter[:], kc(l, 4),
                                            acc7[:], op.add, op.max)
                vector.scalar_tensor_tensor(acc3[:], inter[:], kc(l, 5),
                                            acc3[:], op.add,
                                            op.max).then_inc(v_sem, 1)
                if l > 0:
                    vector.wait_ge(a_sem, 2 * l)
                    vector.tensor_tensor(colv[:], lnI[(l - 1) % 2][:],
                                         lnD[(l - 1) % 2][:], op.subtract)
                    vector.tensor_reduce(colmax[:, l - 1:l], colv[:],
                                         mybir.AxisListType.X, op.max)
            l = L - 1
            vector.wait_ge(a_sem, 2 * L)
            vector.tensor_tensor(colv[:], lnI[l % 2][:], lnD[l % 2][:],
                                 op.subtract)
            vector.tensor_reduce(colmax[:, l:l + 1], colv[:],
                                 mybir.AxisListType.X, op.max)
            vector.scalar_tensor_tensor(acc7[:], sb["area"][:], -Q7, acc7[:],
                                        op.mult, op.add)
            vector.scalar_tensor_tensor(acc3[:], sb["area"][:], -Q3, acc3[:],
                                        op.mult, op.add).then_inc(f_sem, 1)

    return nc


def _iou_rows_exact(anchors, valid, gt, idxs):
    """Reference-exact f32 masked iou rows for the given anchor indices."""
    a = anchors[idxs]
    tl = np.maximum(a[:, None, :2], gt[None, :, :2])
    br = np.minimum(a[:, None, 2:], gt[None, :, 2:])
    wh = np.clip(br - tl, 0.0, None).astype(f32)
    inter = (wh[..., 0] * wh[..., 1]).astype(f32)
    area_a = ((a[:, 2] - a[:, 0]) * (a[:, 3] - a[:, 1])).astype(f32)
    area_b = ((gt[:, 2] - gt[:, 0]) * (gt[:, 3] - gt[:, 1])).astype(f32)
    denom = (((area_a[:, None] + area_b[None, :]).astype(f32) - inter).astype(f32)
             + f32(1e-8)).astype(f32)
    iou = (inter / denom).astype(f32)
    return np.where(valid[idxs][:, None], iou, f32(-1.0))


def kernel(anchors, gt_bboxes, image_shape):
    anchors = np.asarray(anchors, f32)
    gt = np.asarray(gt_bboxes, f32)
    ish = np.asarray(image_shape)
    h = f32(float(ish[0]))
    w = f32(float(ish[1]))

    valid = ((anchors[:, 0] >= 0) & (anchors[:, 1] >= 0) &
             (anchors[:, 2] <= h - f32(1.0)) & (anchors[:, 3] <= w - f32(1.0)))

    # --- spatial sharding: y-strips of valid anchors with boundaries chosen
    # to minimize the max per-strip gt count (invalid anchors are filler).
    # A gt is kept for a strip iff some strip anchor could reach iou >= 0.3:
    # iou <= hy_max/gh, so hy_max <= 0.299*gh proves iou < 0.3 (exact cull).
    vid = np.flatnonzero(valid)
    iid = np.flatnonzero(~valid)
    cyv = (anchors[vid, 0] + anchors[vid, 2]) * f32(0.5)
    vid = vid[np.argsort(cyv, kind="stable")]
    nv = len(vid)
    ay0 = anchors[vid, 0].astype(np.float64)
    ay2 = anchors[vid, 2].astype(np.float64)

    def gtcount_ext(lo, hi):
        cnt = 0
        for g in range(G):
            hy = min(hi, float(gt[g, 2])) - max(lo, float(gt[g, 0]))
            if hy > 0.299 * (float(gt[g, 2]) - float(gt[g, 0])):
                cnt += 1
        return cnt

    def carve(T):
        start = 0
        bounds = []
        for c in range(NC):
            if start >= nv:
                bounds.append((start, start))
                continue
            end = min(start + SHARD, nv)
            amin = np.minimum.accumulate(ay0[start:end])
            amax = np.maximum.accumulate(ay2[start:end])
            lo_i, hi_i, best = 1, end - start, 1
            while lo_i <= hi_i:
                mid = (lo_i + hi_i) // 2
                if gtcount_ext(amin[mid - 1], amax[mid - 1]) <= T:
                    best = mid
                    lo_i = mid + 1
                else:
                    hi_i = mid - 1
            bounds.append((start, start + best))
            start += best
        return start >= nv, bounds

    lo_t, hi_t = 1, G
    while lo_t < hi_t:
        mid = (lo_t + hi_t) // 2
        if carve(mid)[0]:
            hi_t = mid
        else:
            lo_t = mid + 1
    ok, bounds = carve(lo_t)
    assert ok

    a = anchors.copy()
    degen = np.array([1e6, 1e6, -1e6, -1e6], f32)
    a[~valid] = degen

    perm_parts = []
    core_lists = []
    ipos = 0
    for c in range(NC):
        s_, e_ = bounds[c]
        ids = vid[s_:e_]
        fill = SHARD - len(ids)
        if fill:
            ids = np.concatenate([ids, iid[ipos:ipos + fill]])
            ipos += fill
        perm_parts.append(ids)
        if e_ > s_:
            lo = float(ay0[s_:e_].min())
            hi = float(ay2[s_:e_].max())
            core_lists.append(
                [g for g in range(G)
                 if (min(hi, float(gt[g, 2])) - max(lo, float(gt[g, 0])))
                 > 0.299 * (float(gt[g, 2]) - float(gt[g, 0]))])
        else:
            core_lists.append([])
    assert ipos == len(iid)
    perm = np.concatenate(perm_parts)
    a_sorted = a[perm]
    valid_sorted = valid[perm]
    L = max(1, max(len(lst) for lst in core_lists))

    cg_all = [np.float64(f32(gt[g, 2] - gt[g, 0]) * f32(gt[g, 3] - gt[g, 1]))
              + 1e-8 for g in range(G)]
    in_maps = []
    for c in range(NC):
        sh = np.concatenate([a_sorted[c * SHARD:(c + 1) * SHARD],
                             np.tile(degen[None], (PAD, 1))])
        aymin, axmin, aymax, axmax = sh[:, 0], sh[:, 1], sh[:, 2], sh[:, 3]
        krow = np.empty(7 * L + 1, np.float64)
        for l in range(L):
            if l < len(core_lists[c]):
                g = core_lists[c][l]
                krow[7 * l:7 * l + 7] = [gt[g, 2], -gt[g, 0], gt[g, 3],
                                         -gt[g, 1], -Q7 * cg_all[g],
                                         -Q3 * cg_all[g], cg_all[g]]
            else:
                krow[7 * l:7 * l + 7] = [-1e7, 1e7, -1e7, 1e7, -1e9, -1e9, 1e9]
        krow[7 * L] = 1e-10
        in_maps.append({
            "aymax": np.ascontiguousarray(aymax.reshape(P, C)),
            "naymin": np.ascontiguousarray((-aymin).reshape(P, C)),
            "axmax": np.ascontiguousarray(axmax.reshape(P, C)),
            "naxmin": np.ascontiguousarray((-axmin).reshape(P, C)),
            "area": np.ascontiguousarray(
                ((aymax - aymin) * (axmax - axmin)).astype(f32).reshape(P, C)),
            "consts": np.ascontiguousarray(
                np.broadcast_to(krow.astype(f32), (P, 7 * L + 1))),
        })

    if L not in _PROG_CACHE:
        _PROG_CACHE[L] = _build_program_v3(L)
    nc = _PROG_CACHE[L]

    res = run_bass_kernel_spmd(nc, in_maps, core_ids=list(range(NC)))
    p7_s = np.concatenate(
        [res.results[c]["p7"].reshape(-1)[:SHARD] for c in range(NC)])
    p3_s = np.concatenate(
        [res.results[c]["p3"].reshape(-1)[:SHARD] for c in range(NC)])
    colmax_all = np.nan_to_num(
        np.stack([res.results[c]["colmax"] for c in range(NC)]), nan=-np.inf)

    # --- host final stage (tiny) ---
    rp, rn, hi_idx = _get_rand()
    labels_s = np.full(N, -1, np.int32)
    labels_s[valid_sorted & (p7_s >= 0)] = 1
    labels_s[valid_sorted & (p3_s < 0)] = 0
    labels = np.empty(N, np.int32)
    labels[perm] = labels_s

    gt_argmax = np.zeros(G, np.int32)
    for g in range(G):
        best = None
        for c in range(NC):
            if g not in core_lists[c]:
                continue
            l = core_lists[c].index(g)
            col = colmax_all[c][:, l]
            p_ = int(np.argmax(col))
            v = col[p_]
            if best is None or v > best[0]:
                best = (v, c, p_)
        assert best is not None, f"gt {g} culled everywhere"
        _, c_, p_ = best
        base = p_ * C
        n_row = min(C, SHARD - base)
        idxs = perm[c_ * SHARD + base + np.arange(n_row)]
        iou = _iou_rows_exact(anchors, valid, gt, idxs)[:, g]
        gt_argmax[g] = idxs[int(np.argmax(iou))]

    labels[gt_argmax] = 1

    pos_all = np.flatnonzero(labels == 1)
    amax = np.zeros(N, np.int32)
    if len(pos_all):
        amax[pos_all] = _iou_rows_exact(anchors, valid, gt, pos_all)\
            .argmax(axis=1).astype(np.int32)
    amax[gt_argmax] = np.arange(G, dtype=np.int32)

    total_pos = int(len(pos_all))
    total_neg = int((labels == 0).sum())
    cur_pos = min(total_pos, MAX_POS)
    cur_neg = min(TOTAL_SAMPLES - cur_pos, total_neg)

    order = np.argsort(-rp[pos_all], kind="stable")
    pos_sorted = pos_all[order]
    if total_pos >= MAX_POS:
        pos_idx = pos_sorted[:MAX_POS].astype(np.int64)
    else:
        nonpos = np.flatnonzero(labels != 1)[:MAX_POS - total_pos]
        pos_idx = np.concatenate([pos_sorted, nonpos]).astype(np.int64)

    negcand = hi_idx[labels[hi_idx] == 0]
    if len(negcand) < cur_neg:
        negcand = np.flatnonzero(labels == 0)
    order = np.argsort(-rn[negcand], kind="stable")
    neg_sorted = negcand[order][:cur_neg]

    slot = np.arange(TOTAL_SAMPLES)
    pos_slot = slot < cur_pos
    neg_slot = (slot >= cur_pos) & (slot < cur_pos + cur_neg)
    tai = np.full(TOTAL_SAMPLES, -1, np.int32)
    tai[pos_slot] = pos_idx[slot[pos_slot]]
    tai[neg_slot] = neg_sorted[slot[neg_slot] - cur_pos]
    tl = np.where(pos_slot, 1, np.where(neg_slot, 0, -1)).astype(np.int32)

    sa = anchors[pos_idx]
    sg = gt[amax[pos_idx]]
    ah = sa[:, 2] - sa[:, 0]
    aw = sa[:, 3] - sa[:, 1]
    ay = sa[:, 0] + f32(0.5) * ah
    ax = sa[:, 1] + f32(0.5) * aw
    gh = sg[:, 2] - sg[:, 0]
    gw = sg[:, 3] - sg[:, 1]
    gy = sg[:, 0] + f32(0.5) * gh
    gx = sg[:, 1] + f32(0.5) * gw
    reg = np.stack([(gy - ay) / ah, (gx - ax) / aw,
                    np.log(gh / ah), np.log(gw / aw)], axis=1).astype(f32)
    reg[np.arange(MAX_POS) >= cur_pos] = 0.0

    return (tai, tl, reg, np.int32(cur_pos))


# revision 21
# speedup vs baseline: 1.7480x; 1.7480x over previous
"""RPN training-proposal kernel for Trainium2 (8 NeuronCores, SPMD row-parallel).

Device (per core, anchors row-sharded): for each of 64 gt boxes, computes
J = inter/(area_a + area_g + 1e-8) per anchor — a monotone transform of IoU
(iou = J/(1-J)) — tracking max-over-gt J per anchor (labels) and max-over-
anchor J per gt (per-gt argmax location). Host: threshold labeling, exact
tiny recomputes for the <=1.5k selected/forced anchors, and the 256-sample
top-k selection using the fixed PRNG streams.
"""
import numpy as np

import concourse.bass as bass
import concourse.mybir as mybir
from concourse.bass_utils import run_bass_kernel_spmd

N, G = 500000, 64
NC = 8
SHARD = N // NC           # 62500
P, C = 128, 489           # P*C = 62592 >= SHARD
PAD = P * C - SHARD
TOTAL_SAMPLES, MAX_POS = 256, 128
NEG_T = 0.995             # conservative rn threshold for neg top-k candidates

f32 = np.float32
_PROG_CACHE = {}
_RAND_CACHE = {}


def _get_rand():
    if "rp" not in _RAND_CACHE:
        import jax
        cpu = jax.devices("cpu")[0]
        with jax.default_device(cpu):
            kp, kn = jax.random.split(jax.random.key(42))
            rp = np.asarray(jax.random.uniform(kp, (N,)))
            rn = np.asarray(jax.random.uniform(kn, (N,)))
        _RAND_CACHE["rp"] = rp
        _RAND_CACHE["rn"] = rn
        _RAND_CACHE["hi"] = np.flatnonzero(rn > NEG_T)
    return _RAND_CACHE["rp"], _RAND_CACHE["rn"], _RAND_CACHE["hi"]


Q7 = 0.7 / 1.7
Q3 = 0.3 / 1.3


def _build_program(gt):
    """Bass program with gt constants baked as immediates. gt: [G,4] f32.

    Per gt: inter[p,c] f32-exact; label predicates accumulate
    acc_q = max_g(inter - q*cg) (host compares vs q*area); per-gt argmax
    location via colmax[:,g] = max_c(ln(inter+1e-10) - ln(area+cg)) — a
    monotone transform of iou within the column.
    """
    dt = mybir.dt.float32
    nc = bass.Bass()
    names = ["aymax", "naymin", "axmax", "naxmin", "area"]
    dins = {n: nc.dram_tensor(n, [P, C], dt, kind="ExternalInput") for n in names}
    d_p7 = nc.dram_tensor("p7", [P, C], dt, kind="ExternalOutput")
    d_p3 = nc.dram_tensor("p3", [P, C], dt, kind="ExternalOutput")
    d_colmax = nc.dram_tensor("colmax", [P, G], dt, kind="ExternalOutput")

    gy1 = [float(gt[g, 0]) for g in range(G)]
    gx1 = [float(gt[g, 1]) for g in range(G)]
    gy2 = [float(gt[g, 2]) for g in range(G)]
    gx2 = [float(gt[g, 3]) for g in range(G)]
    cg = [float((f32(gt[g, 2] - gt[g, 0]) * f32(gt[g, 3] - gt[g, 1])).astype(f32)
                + f32(1e-8)) for g in range(G)]

    import contextlib
    with contextlib.ExitStack() as ctx:
        # [P,1] bias columns for the ACT Ln ops: col 0 = 1e-10, col 1+g = cg[g]
        bias_t = ctx.enter_context(nc.sbuf_tensor("ln_biases", [P, G + 1], dt))
        nc.gpsimd.memset(bias_t[:, 0:1], 1e-10)
        for g in range(G):
            nc.gpsimd.memset(bias_t[:, g + 1:g + 2], cg[g])
        nc.all_engine_barrier()
        sb = {n: ctx.enter_context(nc.sbuf_tensor("sb_" + n, [P, C], dt))
              for n in names}
        t1 = ctx.enter_context(nc.sbuf_tensor("t1", [P, C], dt))
        hy = ctx.enter_context(nc.sbuf_tensor("hy", [P, C], dt))
        hyr = ctx.enter_context(nc.sbuf_tensor("hyr", [P, C], dt))
        wx = ctx.enter_context(nc.sbuf_tensor("wx", [P, C], dt))
        inter = ctx.enter_context(nc.sbuf_tensor("inter", [P, C], dt))
        acc7 = ctx.enter_context(nc.sbuf_tensor("acc7", [P, C], dt))
        acc3 = ctx.enter_context(nc.sbuf_tensor("acc3", [P, C], dt))
        lnI = [ctx.enter_context(nc.sbuf_tensor(f"lnI{i}", [P, C], dt))
               for i in range(2)]
        lnD = [ctx.enter_context(nc.sbuf_tensor(f"lnD{i}", [P, C], dt))
               for i in range(2)]
        colv = ctx.enter_context(nc.sbuf_tensor("colv", [P, C], dt))
        colmax = ctx.enter_context(nc.sbuf_tensor("colmax_sb", [P, G], dt))
        dma_sem = ctx.enter_context(nc.semaphore("dma_sem"))
        v_sem = ctx.enter_context(nc.semaphore("v_sem"))
        a_sem = ctx.enter_context(nc.semaphore("a_sem"))
        f_sem = ctx.enter_context(nc.semaphore("f_sem"))
        block = ctx.enter_context(nc.Block())

        op = mybir.AluOpType
        AF = mybir.ActivationFunctionType

        @block.sync
        def _(sync):
            for n in names:
                sync.dma_start(sb[n][:], dins[n][:]).then_inc(dma_sem, 16)
            sync.wait_ge(f_sem, 1)
            sync.dma_start(d_p7[:], acc7[:]).then_inc(dma_sem, 16)
            sync.dma_start(d_p3[:], acc3[:]).then_inc(dma_sem, 16)
            sync.wait_ge(g2_sem, 1)
            sync.dma_start(d_colmax[:], colmax[:]).then_inc(dma_sem, 16)
            sync.wait_ge(dma_sem, 8 * 16)

        @block.scalar
        def _(scalar):
            for g in range(G):
                scalar.wait_ge(v_sem, g + 1)
                scalar.activation(lnI[g % 2][:], inter[:], AF.Ln,
                                  bias=bias_t[:, 0:1]).then_inc(a_sem, 1)
                scalar.activation(lnD[g % 2][:], sb["area"][:], AF.Ln,
                                  bias=bias_t[:, g + 1:g + 2]).then_inc(a_sem, 1)

        @block.vector
        def _(vector):
            vector.memset(acc7[:], -1e30)
            vector.memset(acc3[:], -1e30)
            vector.wait_ge(dma_sem, 5 * 16)
            for g in range(G):
                vector.tensor_scalar_min(t1[:], sb["aymax"][:], gy2[g])
                vector.scalar_tensor_tensor(hy[:], sb["naymin"][:], -gy1[g], t1[:],
                                            op.min, op.add)
                vector.tensor_scalar_max(hyr[:], hy[:], 0.0)
                vector.tensor_scalar_min(t1[:], sb["axmax"][:], gx2[g])
                vector.scalar_tensor_tensor(wx[:], sb["naxmin"][:], -gx1[g], t1[:],
                                            op.min, op.add)
                if g > 0:
                    # inter(g) overwrite must wait for ACT lnI(g-1) read
                    vector.wait_ge(a_sem, 2 * g - 1)
                vector.scalar_tensor_tensor(inter[:], wx[:], 0.0, hyr[:],
                                            op.max, op.mult)
                vector.scalar_tensor_tensor(acc7[:], inter[:], -Q7 * cg[g],
                                            acc7[:], op.add, op.max)
                vector.scalar_tensor_tensor(acc3[:], inter[:], -Q3 * cg[g],
                                            acc3[:], op.add,
                                            op.max).then_inc(v_sem, 1)
                if g > 0:
                    vector.wait_ge(a_sem, 2 * g)
                    vector.tensor_tensor(colv[:], lnI[(g - 1) % 2][:],
                                         lnD[(g - 1) % 2][:], op.subtract)
                    vector.tensor_reduce(colmax[:, g - 1:g], colv[:],
                                         mybir.AxisListType.X, op.max)
            g = G - 1
            vector.wait_ge(a_sem, 2 * G)
            vector.tensor_tensor(colv[:], lnI[g % 2][:], lnD[g % 2][:],
                                 op.subtract)
            vector.tensor_reduce(colmax[:, g:g + 1], colv[:],
                                 mybir.AxisListType.X, op.max)
            # fold q*area into the accumulators: p = acc - q*area
            vector.scalar_tensor_tensor(acc7[:], sb["area"][:], -Q7, acc7[:],
                                        op.mult, op.add)
            vector.scalar_tensor_tensor(acc3[:], sb["area"][:], -Q3, acc3[:],
                                        op.mult, op.add).then_inc(f_sem, 1)

    return nc


N_DVE_GTS = 35  # gts 0..34 on the vector engine, 35..63 on gpsimd


def _build_program_v2(gt):
    """3-engine split: DVE and GPSIMD each own a gt subset (full predicate
    chain + colv); ACT computes the two Ln's per gt; DVE does all column
    reduces and the final merge."""
    dt = mybir.dt.float32
    nc = bass.Bass()
    names = ["aymax", "naymin", "axmax", "naxmin", "area"]
    dins = {n: nc.dram_tensor(n, [P, C], dt, kind="ExternalInput") for n in names}
    d_p7 = nc.dram_tensor("p7", [P, C], dt, kind="ExternalOutput")
    d_p3 = nc.dram_tensor("p3", [P, C], dt, kind="ExternalOutput")
    d_colmax = nc.dram_tensor("colmax", [P, G], dt, kind="ExternalOutput")

    gy1 = [float(gt[g, 0]) for g in range(G)]
    gx1 = [float(gt[g, 1]) for g in range(G)]
    gy2 = [float(gt[g, 2]) for g in range(G)]
    gx2 = [float(gt[g, 3]) for g in range(G)]
    cg = [float((f32(gt[g, 2] - gt[g, 0]) * f32(gt[g, 3] - gt[g, 1])).astype(f32)
                + f32(1e-8)) for g in range(G)]
    DG = list(range(N_DVE_GTS))
    SG = list(range(N_DVE_GTS, G))

    import contextlib
    with contextlib.ExitStack() as ctx:
        bias_t = ctx.enter_context(nc.sbuf_tensor("ln_biases", [P, G + 1], dt))
        nc.gpsimd.memset(bias_t[:, 0:1], 1e-10)
        for g in range(G):
            nc.gpsimd.memset(bias_t[:, g + 1:g + 2], cg[g])
        nc.all_engine_barrier()
        sb = {n: ctx.enter_context(nc.sbuf_tensor("sb_" + n, [P, C], dt))
              for n in names}

        def sbufs(prefix, num=1):
            if num == 1:
                return ctx.enter_context(nc.sbuf_tensor(prefix, [P, C], dt))
            return [ctx.enter_context(nc.sbuf_tensor(f"{prefix}{i}", [P, C], dt))
                    for i in range(num)]

        t1D, hyD, hyrD, wxD, interD = (sbufs(x) for x in
                                       ("t1D", "hyD", "hyrD", "wxD", "interD"))
        t1S, hyS, hyrS, wxS, interS = (sbufs(x) for x in
                                       ("t1S", "hyS", "hyrS", "wxS", "interS"))
        acc7D, acc3D, acc7S, acc3S = (sbufs(x) for x in
                                      ("acc7D", "acc3D", "acc7S", "acc3S"))
        lnID, lnDD = sbufs("lnID", 2), sbufs("lnDD", 2)
        lnIS, lnDS = sbufs("lnIS", 2), sbufs("lnDS", 2)
        colvD = sbufs("colvD")
        colvS = sbufs("colvS", len(SG))
        colmax = ctx.enter_context(nc.sbuf_tensor("colmax_sb", [P, G], dt))
        dma_sem = ctx.enter_context(nc.semaphore("dma_sem"))
        vD = ctx.enter_context(nc.semaphore("vD"))
        vS = ctx.enter_context(nc.semaphore("vS"))
        aD = ctx.enter_context(nc.semaphore("aD"))
        aS = ctx.enter_context(nc.semaphore("aS"))
        cS = ctx.enter_context(nc.semaphore("cS"))
        g_done = ctx.enter_context(nc.semaphore("g_done"))
        f_sem = ctx.enter_context(nc.semaphore("f_sem"))
        block = ctx.enter_context(nc.Block())

        op = mybir.AluOpType
        AF = mybir.ActivationFunctionType

        @block.sync
        def _(sync):
            for n in names:
                sync.dma_start(sb[n][:], dins[n][:]).then_inc(dma_sem, 16)
            sync.wait_ge(f_sem, 1)
            sync.dma_start(d_p7[:], acc7D[:]).then_inc(dma_sem, 16)
            sync.dma_start(d_p3[:], acc3D[:]).then_inc(dma_sem, 16)
            sync.dma_start(d_colmax[:], colmax[:]).then_inc(dma_sem, 16)
            sync.wait_ge(dma_sem, 8 * 16)

        @block.scalar
        def _(scalar):
            for i in range(max(len(DG), len(SG))):
                if i < len(DG):
                    g = DG[i]
                    scalar.wait_ge(vD, i + 1)
                    scalar.activation(lnID[i % 2][:], interD[:], AF.Ln,
                                      bias=bias_t[:, 0:1]).then_inc(aD, 1)
                    scalar.activation(lnDD[i % 2][:], sb["area"][:], AF.Ln,
                                      bias=bias_t[:, g + 1:g + 2]).then_inc(aD, 1)
                if i < len(SG):
                    g = SG[i]
                    scalar.wait_ge(vS, i + 1)
                    scalar.activation(lnIS[i % 2][:], interS[:], AF.Ln,
                                      bias=bias_t[:, 0:1]).then_inc(aS, 1)
                    scalar.activation(lnDS[i % 2][:], sb["area"][:], AF.Ln,
                                      bias=bias_t[:, g + 1:g + 2]).then_inc(aS, 1)

        def chain(eng, i, g, t1, hy, hyr, wx, inter, acc7, acc3, a_sem, v_sem):
            eng.tensor_scalar_min(t1[:], sb["aymax"][:], gy2[g])
            eng.scalar_tensor_tensor(hy[:], sb["naymin"][:], -gy1[g], t1[:],
                                     op.min, op.add)
            eng.tensor_scalar_max(hyr[:], hy[:], 0.0)
            eng.tensor_scalar_min(t1[:], sb["axmax"][:], gx2[g])
            eng.scalar_tensor_tensor(wx[:], sb["naxmin"][:], -gx1[g], t1[:],
                                     op.min, op.add)
            if i > 0:
                eng.wait_ge(a_sem, 2 * i - 1)
            eng.scalar_tensor_tensor(inter[:], wx[:], 0.0, hyr[:],
                                     op.max, op.mult)
            eng.scalar_tensor_tensor(acc7[:], inter[:], -Q7 * cg[g], acc7[:],
                                     op.add, op.max)
            eng.scalar_tensor_tensor(acc3[:], inter[:], -Q3 * cg[g], acc3[:],
                                     op.add, op.max).then_inc(v_sem, 1)

        @block.gpsimd
        def _(gpsimd):
            gpsimd.memset(acc7S[:], -1e30)
            gpsimd.memset(acc3S[:], -1e30)
            gpsimd.wait_ge(dma_sem, 5 * 16)
            for i, g in enumerate(SG):
                chain(gpsimd, i, g, t1S, hyS, hyrS, wxS, interS,
                      acc7S, acc3S, aS, vS)
                if i > 0:
                    gpsimd.wait_ge(aS, 2 * i)
                    gpsimd.scalar_tensor_tensor(
                        colvS[i - 1][:], lnIS[(i - 1) % 2][:], 0.0,
                        lnDS[(i - 1) % 2][:], op.add,
                        op.subtract).then_inc(cS, 1)
            i = len(SG) - 1
            gpsimd.wait_ge(aS, 2 * len(SG))
            gpsimd.scalar_tensor_tensor(
                colvS[i][:], lnIS[i % 2][:], 0.0, lnDS[i % 2][:],
                op.add, op.subtract).then_inc(cS, 1)
            gpsimd.engine_nop().then_inc(g_done, 1)

        @block.vector
        def _(vector):
            vector.memset(acc7D[:], -1e30)
            vector.memset(acc3D[:], -1e30)
            vector.wait_ge(dma_sem, 5 * 16)
            for i, g in enumerate(DG):
                chain(vector, i, g, t1D, hyD, hyrD, wxD, interD,
                      acc7D, acc3D, aD, vD)
                if i > 0:
                    vector.wait_ge(aD, 2 * i)
                    vector.tensor_tensor(colvD[:], lnID[(i - 1) % 2][:],
                                         lnDD[(i - 1) % 2][:], op.subtract)
                    vector.tensor_reduce(colmax[:, DG[i - 1]:DG[i - 1] + 1],
                                         colvD[:], mybir.AxisListType.X, op.max)
            i = len(DG) - 1
            vector.wait_ge(aD, 2 * len(DG))
            vector.tensor_tensor(colvD[:], lnID[i % 2][:], lnDD[i % 2][:],
                                 op.subtract)
            vector.tensor_reduce(colmax[:, DG[i]:DG[i] + 1], colvD[:],
                                 mybir.AxisListType.X, op.max)
            for j in range(len(SG)):
                vector.wait_ge(cS, j + 1)
                vector.tensor_reduce(colmax[:, SG[j]:SG[j] + 1],
                                     colvS[j][:], mybir.AxisListType.X, op.max)
            vector.wait_ge(g_done, 1)
            vector.tensor_tensor(acc7D[:], acc7D[:], acc7S[:], op.max)
            vector.tensor_tensor(acc3D[:], acc3D[:], acc3S[:], op.max)
            vector.scalar_tensor_tensor(acc7D[:], sb["area"][:], -Q7, acc7D[:],
                                        op.mult, op.add)
            vector.scalar_tensor_tensor(acc3D[:], sb["area"][:], -Q3, acc3D[:],
                                        op.mult, op.add).then_inc(f_sem, 1)

    return nc


def _build_program_v3(L, Cc):
    """Culled variant: L gt-slots per core, per-slot constants supplied via a
    [P, 7L+1] input ("consts"), so the program depends only on L. Slot l
    columns: [gy2, -gy1, gx2, -gx1, -q7*cg, -q3*cg, cg]; column 7L = 1e-10
    (the Ln epsilon)."""
    dt = mybir.dt.float32
    nc = bass.Bass()
    names = ["aymax", "naymin", "axmax", "naxmin", "area"]
    dins = {n: nc.dram_tensor(n, [P, Cc], dt, kind="ExternalInput") for n in names}
    d_consts = nc.dram_tensor("consts", [P, 7 * L + 1], dt, kind="ExternalInput")
    d_p7 = nc.dram_tensor("p7", [P, Cc], dt, kind="ExternalOutput")
    d_p3 = nc.dram_tensor("p3", [P, Cc], dt, kind="ExternalOutput")
    d_colmax = nc.dram_tensor("colmax", [1, L * Cc], dt, kind="ExternalOutput")

    import contextlib
    with contextlib.ExitStack() as ctx:
        ko = ctx.enter_context(nc.sbuf_tensor("ko", [P, 7 * L + 1], dt))
        sb = {n: ctx.enter_context(nc.sbuf_tensor("sb_" + n, [P, Cc], dt))
              for n in names}
        t1y = [ctx.enter_context(nc.sbuf_tensor(f"t1y{i}", [P, Cc], dt))
               for i in range(4)]
        t1x = [ctx.enter_context(nc.sbuf_tensor(f"t1x{i}", [P, Cc], dt))
               for i in range(4)]
        hy = ctx.enter_context(nc.sbuf_tensor("hy", [P, Cc], dt))
        hyr = ctx.enter_context(nc.sbuf_tensor("hyr", [P, Cc], dt))
        wx = ctx.enter_context(nc.sbuf_tensor("wx", [P, Cc], dt))
        inter = ctx.enter_context(nc.sbuf_tensor("inter", [P, Cc], dt))
        acc7 = ctx.enter_context(nc.sbuf_tensor("acc7", [P, Cc], dt))
        acc3 = ctx.enter_context(nc.sbuf_tensor("acc3", [P, Cc], dt))
        lnI = [ctx.enter_context(nc.sbuf_tensor(f"lnI{i}", [P, Cc], dt))
               for i in range(2)]
        lnD = [ctx.enter_context(nc.sbuf_tensor(f"lnD{i}", [P, Cc], dt))
               for i in range(2)]
        colv = [ctx.enter_context(nc.sbuf_tensor(f"colv{i}", [P, Cc], dt))
                for i in range(4)]
        colmax = ctx.enter_context(nc.sbuf_tensor("colmax_sb", [1, L * Cc], dt))
        dma_sem = ctx.enter_context(nc.semaphore("dma_sem"))
        v_sem = ctx.enter_context(nc.semaphore("v_sem"))
        a_sem = ctx.enter_context(nc.semaphore("a_sem"))
        gt1_sem = ctx.enter_context(nc.semaphore("gt1_sem"))
        cv_sem = ctx.enter_context(nc.semaphore("cv_sem"))
        gR_sem = ctx.enter_context(nc.semaphore("gR_sem"))
        g2_sem = ctx.enter_context(nc.semaphore("g2_sem"))
        f_sem = ctx.enter_context(nc.semaphore("f_sem"))
        block = ctx.enter_context(nc.Block())

        op = mybir.AluOpType
        AF = mybir.ActivationFunctionType

        def kc(l, j):
            return ko[:, 7 * l + j:7 * l + j + 1]

        @block.sync
        def _(sync):
            sync.dma_start(ko[:], d_consts[:]).then_inc(dma_sem, 16)
            for n in names:
                sync.dma_start(sb[n][:], dins[n][:]).then_inc(dma_sem, 16)
            sync.wait_ge(f_sem, 1)
            sync.dma_start(d_p7[:], acc7[:]).then_inc(dma_sem, 16)
            sync.dma_start(d_p3[:], acc3[:]).then_inc(dma_sem, 16)
            sync.wait_ge(g2_sem, 1)
            sync.dma_start(d_colmax[:], colmax[:]).then_inc(dma_sem, 16)
            sync.wait_ge(dma_sem, 9 * 16)

        @block.gpsimd
        def _(gpsimd):
            gpsimd.wait_ge(dma_sem, 4 * 16)
            for l in range(L):
                if l >= 4:
                    gpsimd.wait_ge(v_sem, l - 3)
                gpsimd.tensor_scalar_min(t1y[l % 4][:], sb["aymax"][:],
                                         kc(l, 0))
                gpsimd.tensor_scalar_min(t1x[l % 4][:], sb["axmax"][:],
                                         kc(l, 2)).then_inc(gt1_sem, 1)
                j = l - 4
                if j >= 0:
                    gpsimd.wait_ge(cv_sem, j + 1)
                    gpsimd.tensor_reduce(
                        colmax[0:1, j * Cc:(j + 1) * Cc], colv[j % 4][:],
                        mybir.AxisListType.C, op.max).then_inc(gR_sem, 1)
            for j in range(max(0, L - 4), L):
                gpsimd.wait_ge(cv_sem, j + 1)
                ins = gpsimd.tensor_reduce(
                    colmax[0:1, j * Cc:(j + 1) * Cc], colv[j % 4][:],
                    mybir.AxisListType.C, op.max)
                ins.then_inc(gR_sem, 1)
            gpsimd.engine_nop().then_inc(g2_sem, 1)

        @block.scalar
        def _(scalar):
            for l in range(L):
                scalar.wait_ge(v_sem, l + 1)
                scalar.activation(lnI[l % 2][:], inter[:], AF.Ln,
                                  bias=ko[:, 7 * L:7 * L + 1]).then_inc(a_sem, 1)
                scalar.activation(lnD[l % 2][:], sb["area"][:], AF.Ln,
                                  bias=kc(l, 6)).then_inc(a_sem, 1)

        @block.vector
        def _(vector):
            vector.memset(acc7[:], -1e30)
            vector.memset(acc3[:], -1e30)
            vector.wait_ge(dma_sem, 3 * 16)
            for l in range(L):
                vector.wait_ge(gt1_sem, l + 1)
                vector.scalar_tensor_tensor(hy[:], sb["naymin"][:], kc(l, 1),
                                            t1y[l % 4][:], op.min, op.add)
                vector.tensor_scalar_max(hyr[:], hy[:], 0.0)
                if l == 0:
                    vector.wait_ge(dma_sem, 6 * 16)
                vector.scalar_tensor_tensor(wx[:], sb["naxmin"][:], kc(l, 3),
                                            t1x[l % 4][:], op.min, op.add)
                if l > 0:
                    vector.wait_ge(a_sem, 2 * l - 1)
                vector.scalar_tensor_tensor(inter[:], wx[:], 0.0, hyr[:],
                                            op.max, op.mult)
                vector.scalar_tensor_tensor(acc7[:], inter[:], kc(l, 4),
                                            acc7[:], op.add, op.max)
                vector.scalar_tensor_tensor(acc3[:], inter[:], kc(l, 5),
                                            acc3[:], op.add,
                                            op.max).then_inc(v_sem, 1)
                if l > 0:
                    vector.wait_ge(a_sem, 2 * l)
                    if l > 4:
                        vector.wait_ge(gR_sem, l - 4)
                    vector.tensor_tensor(colv[(l - 1) % 4][:],
                                         lnI[(l - 1) % 2][:],
                                         lnD[(l - 1) % 2][:],
                                         op.subtract).then_inc(cv_sem, 1)
            l = L - 1
            vector.wait_ge(a_sem, 2 * L)
            if L > 3:
                vector.wait_ge(gR_sem, L - 3)
            vector.tensor_tensor(colv[l % 4][:], lnI[l % 2][:], lnD[l % 2][:],
                                 op.subtract).then_inc(cv_sem, 1)
            vector.scalar_tensor_tensor(acc7[:], sb["area"][:], -Q7, acc7[:],
                                        op.mult, op.add)
            vector.scalar_tensor_tensor(acc3[:], sb["area"][:], -Q3, acc3[:],
                                        op.mult, op.add).then_inc(f_sem, 1)

    return nc


def _iou_rows_exact(anchors, valid, gt, idxs):
    """Reference-exact f32 masked iou rows for the given anchor indices."""
    a = anchors[idxs]
    tl = np.maximum(a[:, None, :2], gt[None, :, :2])
    br = np.minimum(a[:, None, 2:], gt[None, :, 2:])
    wh = np.clip(br - tl, 0.0, None).astype(f32)
    inter = (wh[..., 0] * wh[..., 1]).astype(f32)
    area_a = ((a[:, 2] - a[:, 0]) * (a[:, 3] - a[:, 1])).astype(f32)
    area_b = ((gt[:, 2] - gt[:, 0]) * (gt[:, 3] - gt[:, 1])).astype(f32)
    denom = (((area_a[:, None] + area_b[None, :]).astype(f32) - inter).astype(f32)
             + f32(1e-8)).astype(f32)
    iou = (inter / denom).astype(f32)
    return np.where(valid[idxs][:, None], iou, f32(-1.0))


def kernel(anchors, gt_bboxes, image_shape):
    anchors = np.asarray(anchors, f32)
    gt = np.asarray(gt_bboxes, f32)
    ish = np.asarray(image_shape)
    h = f32(float(ish[0]))
    w = f32(float(ish[1]))

    valid = ((anchors[:, 0] >= 0) & (anchors[:, 1] >= 0) &
             (anchors[:, 2] <= h - f32(1.0)) & (anchors[:, 3] <= w - f32(1.0)))

    # --- spatial sharding: only VALID anchors go to the device (invalid are
    # labeled -1 directly by the host). Valid anchors are y-sorted and carved
    # into 8 strips; boundaries minimize slots*width cost, where a gt is kept
    # for a strip iff some strip anchor could reach iou >= 0.3
    # (iou <= hy_max/gh, so hy_max <= 0.299*gh proves iou < 0.3 — exact cull).
    vid = np.flatnonzero(valid)
    cyv = (anchors[vid, 0] + anchors[vid, 2]) * f32(0.5)
    vid = vid[np.argsort(cyv, kind="stable")]
    nv = len(vid)
    ay0 = anchors[vid, 0].astype(np.float64)
    ay2 = anchors[vid, 2].astype(np.float64)

    def gtcount_ext(lo, hi):
        cnt = 0
        for g in range(G):
            hy = min(hi, float(gt[g, 2])) - max(lo, float(gt[g, 0]))
            if hy > 0.299 * (float(gt[g, 2]) - float(gt[g, 0])):
                cnt += 1
        return cnt

    def carve(T, V):
        start = 0
        bounds = []
        for c in range(NC):
            if start >= nv:
                bounds.append((start, start))
                continue
            end = min(start + V, nv)
            amin = np.minimum.accumulate(ay0[start:end])
            amax = np.maximum.accumulate(ay2[start:end])
            lo_i, hi_i, best = 1, end - start, 1
            while lo_i <= hi_i:
                mid = (lo_i + hi_i) // 2
                if gtcount_ext(amin[mid - 1], amax[mid - 1]) <= T:
                    best = mid
                    lo_i = mid + 1
                else:
                    hi_i = mid - 1
            bounds.append((start, start + best))
            start += best
        return start >= nv, bounds

    def min_T(V):
        lo_t, hi_t = 1, G
        while lo_t < hi_t:
            mid = (lo_t + hi_t) // 2
            if carve(mid, V)[0]:
                hi_t = mid
            else:
                lo_t = mid + 1
        ok, b = carve(lo_t, V)
        return (lo_t, b) if ok else (None, None)

    v_lo = -(-nv // NC)
    best_cfg = None
    for V in range(v_lo + 100, v_lo + 100 + 8 * 3000, 3000):
        T, b = min_T(V)
        if T is None:
            continue
        w_max = max(e_ - s_ for s_, e_ in b)
        cc = -(-w_max // P)
        cost = T * (8.5 * cc + 1510)
        if best_cfg is None or cost < best_cfg[0]:
            best_cfg = (cost, T, b, cc)
    _, L, bounds, Cc = best_cfg
    L = max(1, L)

    degen = np.array([1e6, 1e6, -1e6, -1e6], f32)
    core_ids_list = []
    core_lists = []
    for c in range(NC):
        s_, e_ = bounds[c]
        core_ids_list.append(vid[s_:e_])
        if e_ > s_:
            lo = float(ay0[s_:e_].min())
            hi = float(ay2[s_:e_].max())
            core_lists.append(
                [g for g in range(G)
                 if (min(hi, float(gt[g, 2])) - max(lo, float(gt[g, 0])))
                 > 0.299 * (float(gt[g, 2]) - float(gt[g, 0]))])
        else:
            core_lists.append([])
    L = max(1, max(len(lst) for lst in core_lists))

    cg_all = [np.float64(f32(gt[g, 2] - gt[g, 0]) * f32(gt[g, 3] - gt[g, 1]))
              + 1e-8 for g in range(G)]
    in_maps = []
    for c in range(NC):
        ids = core_ids_list[c]
        sh = np.concatenate([anchors[ids],
                             np.tile(degen[None], (P * Cc - len(ids), 1))])
        aymin, axmin, aymax, axmax = sh[:, 0], sh[:, 1], sh[:, 2], sh[:, 3]
        krow = np.empty(7 * L + 1, np.float64)
        for l in range(L):
            if l < len(core_lists[c]):
                g = core_lists[c][l]
                krow[7 * l:7 * l + 7] = [gt[g, 2], -gt[g, 0], gt[g, 3],
                                         -gt[g, 1], -Q7 * cg_all[g],
                                         -Q3 * cg_all[g], cg_all[g]]
            else:
                krow[7 * l:7 * l + 7] = [-1e7, 1e7, -1e7, 1e7, -1e9, -1e9, 1e9]
        krow[7 * L] = 1e-10
        in_maps.append({
            "aymax": np.ascontiguousarray(aymax.reshape(P, Cc)),
            "naymin": np.ascontiguousarray((-aymin).reshape(P, Cc)),
            "axmax": np.ascontiguousarray(axmax.reshape(P, Cc)),
            "naxmin": np.ascontiguousarray((-axmin).reshape(P, Cc)),
            "area": np.ascontiguousarray(
                ((aymax - aymin) * (axmax - axmin)).astype(f32).reshape(P, Cc)),
            "consts": np.ascontiguousarray(
                np.broadcast_to(krow.astype(f32), (P, 7 * L + 1))),
        })

    if (L, Cc) not in _PROG_CACHE:
        _PROG_CACHE[(L, Cc)] = _build_program_v3(L, Cc)
    nc = _PROG_CACHE[(L, Cc)]

    res = run_bass_kernel_spmd(nc, in_maps, core_ids=list(range(NC)))
    colmax_all = np.nan_to_num(
        np.stack([res.results[c]["colmax"].reshape(L, Cc) for c in range(NC)]),
        nan=-np.inf)

    # --- host final stage (tiny) ---
    rp, rn, hi_idx = _get_rand()
    labels = np.full(N, -1, np.int32)
    for c in range(NC):
        ids = core_ids_list[c]
        if not len(ids):
            continue
        p7c = res.results[c]["p7"].reshape(-1)[:len(ids)]
        p3c = res.results[c]["p3"].reshape(-1)[:len(ids)]
        labels[ids[p7c >= 0]] = 1
        labels[ids[(p7c < 0) & (p3c < 0)]] = 0

    gt_argmax = np.zeros(G, np.int32)
    for g in range(G):
        best = None
        for c in range(NC):
            if g not in core_lists[c]:
                continue
            l = core_lists[c].index(g)
            row = colmax_all[c][l]
            q_ = int(np.argmax(row))
            v = row[q_]
            if best is None or v > best[0]:
                best = (v, c, q_)
        assert best is not None, f"gt {g} culled everywhere"
        _, c_, q_ = best
        ids = core_ids_list[c_]
        idxs = ids[q_::Cc]
        assert len(idxs) > 0
        iou = _iou_rows_exact(anchors, valid, gt, idxs)[:, g]
        gt_argmax[g] = idxs[int(np.argmax(iou))]

    labels[gt_argmax] = 1

    pos_all = np.flatnonzero(labels == 1)
    amax = np.zeros(N, np.int32)
    if len(pos_all):
        amax[pos_all] = _iou_rows_exact(anchors, valid, gt, pos_all)\
            .argmax(axis=1).astype(np.int32)
    amax[gt_argmax] = np.arange(G, dtype=np.int32)

    total_pos = int(len(pos_all))
    total_neg = int((labels == 0).sum())
    cur_pos = min(total_pos, MAX_POS)
    cur_neg = min(TOTAL_SAMPLES - cur_pos, total_neg)

    order = np.argsort(-rp[pos_all], kind="stable")
    pos_sorted = pos_all[order]
    if total_pos >= MAX_POS:
        pos_idx = pos_sorted[:MAX_POS].astype(np.int64)
    else:
        nonpos = np.flatnonzero(labels != 1)[:MAX_POS - total_pos]
        pos_idx = np.concatenate([pos_sorted, nonpos]).astype(np.int64)

    negcand = hi_idx[labels[hi_idx] == 0]
    if len(negcand) < cur_neg:
        negcand = np.flatnonzero(labels == 0)
    order = np.argsort(-rn[negcand], kind="stable")
    neg_sorted = negcand[order][:cur_neg]

    slot = np.arange(TOTAL_SAMPLES)
    pos_slot = slot < cur_pos
    neg_slot = (slot >= cur_pos) & (slot < cur_pos + cur_neg)
    tai = np.full(TOTAL_SAMPLES, -1, np.int32)
    tai[pos_slot] = pos_idx[slot[pos_slot]]
    tai[neg_slot] = neg_sorted[slot[neg_slot] - cur_pos]
    tl = np.where(pos_slot, 1, np.where(neg_slot, 0, -1)).astype(np.int32)

    sa = anchors[pos_idx]
    sg = gt[amax[pos_idx]]
    ah = sa[:, 2] - sa[:, 0]
    aw = sa[:, 3] - sa[:, 1]
    ay = sa[:, 0] + f32(0.5) * ah
    ax = sa[:, 1] + f32(0.5) * aw
    gh = sg[:, 2] - sg[:, 0]
    gw = sg[:, 3] - sg[:, 1]
    gy = sg[:, 0] + f32(0.5) * gh
    gx = sg[:, 1] + f32(0.5) * gw
    reg = np.stack([(gy - ay) / ah, (gx - ax) / aw,
                    np.log(gh / ah), np.log(gw / aw)], axis=1).astype(f32)
    reg[np.arange(MAX_POS) >= cur_pos] = 0.0

    return (tai, tl, reg, np.int32(cur_pos))
